# revision 25
# baseline (speedup 1.0000x reference)
"""Trainium2 Bass kernel for GQA attention (B=2, S=2048, HID=2048, 16 q-heads,
4 kv-heads, HD=128, RoPE, softmax, output projection).

Sharding: 8 cores = (2 batches) x (4 query-head groups of 4 heads). Each core
owns one batch's hidden states, 4 query heads, and the single kv head those
query heads attend to (GQA group), plus the matching 512-row slice of Wo.
Core (b, g) computes a [S, HID] partial of the output projection (bf16); the
host sums the 4 partials per batch in fp32.

v2 dataflow (all-bf16 operands, fp32 PSUM accumulation):
  KV pass: K^T/V projections for all s-blocks, RoPE on K^T, V transposed to
    key-major chunks via PE.
  Fused main loop per 512-query i-block:
    Q projection + RoPE (per head) -> output projection of the previous
    i-block (software-pipelined) -> attention with a static interleave
    schedule: scores S(h), attn@V A(h-1), denominator-matmul D(h-2) so the
    PE never waits on the exp (ACT) backlog or the DVE esum tree.
  Softmax denominators: bf16 tree-sum of E chunks on DVE (4x mode), one
    all-ones [128,128] matmul per head-block, reciprocal_approx_fast.
  Output projection uses 1024-wide moving operands (PSUM pairs) to halve
  instruction count; results copied to SBUF as bf16 partials and DMA'd out.
"""

import sys
import types

sys.path.insert(0, "/opt/trn_rl_repo")

import numpy as np
import ml_dtypes

B, S, HID = 2, 2048, 2048
NH, NKV, HD = 16, 4, 128
GROUPS = NH // NKV          # q heads per kv head == heads per core
ROPE_THETA = 10000.0
P = 128                     # SBUF partitions
SB = 512                    # s-block (matmul moving dim / psum bank)
N_CORES = 8

BF16 = ml_dtypes.bfloat16

_built = None


def _install_ntff_hook():
    """antenv.axon_hooks is missing from the agent image, which silently
    disables trace=True; recreate it and register the ctypes NTFF hook."""
    if "antenv.axon_hooks" in sys.modules:
        return
    m = types.ModuleType("antenv.axon_hooks")
    m._hook = None
    m.set_axon_ntff_profile_hook = lambda h: setattr(m, "_hook", h)
    m.get_axon_ntff_profile_hook = lambda: m._hook
    sys.modules["antenv.axon_hooks"] = m
    try:
        import antenv

        antenv.axon_hooks = m
    except ImportError:
        pass
    try:
        sys.path.insert(0, "/root/.axon_site/trn_agent_boot")
        from trn_boot import _ntff_profile_via_ctypes

        hook = _ntff_profile_via_ctypes("/opt/axon/libaxon_pjrt.so")
        if hook is not None:
            m.set_axon_ntff_profile_hook(hook)
    except Exception:
        pass


_install_ntff_hook()


def rope_tables():
    """cos table and sign-folded sin table in [HD, S] (transposed) layout.

    sin_signed[d] = -sin for d < HD/2, +sin for d >= HD/2, so RoPE becomes
    out = q * cos + shifted(q) * sin_signed with shifted(q) a partition-half
    swap.
    """
    half = HD // 2
    inv_freq = 1.0 / (ROPE_THETA ** (np.arange(0, HD, 2, dtype=np.float64) / HD))
    t = np.arange(S, dtype=np.float64)
    freqs = np.outer(t, inv_freq)                      # [S, 64]
    emb = np.concatenate([freqs, freqs], axis=-1)      # [S, 128]
    cos_t = np.cos(emb).T.astype(BF16).copy()          # [128, S]
    sin = np.sin(emb).T.astype(np.float32)
    sin_signed = sin.copy()
    sin_signed[:half] *= -1.0
    return cos_t, np.ascontiguousarray(sin_signed.astype(BF16))


def build_bass(s=S, hid=HID):
    import concourse.mybir as mybir
    from concourse import bacc
    from concourse.tile import TileContext

    F32 = mybir.dt.float32
    BF = mybir.dt.bfloat16
    EXP = mybir.ActivationFunctionType.Exp
    MUL = mybir.AluOpType.mult

    kc_n = hid // P            # hid contraction chunks (16)
    sb_n = s // SB             # s blocks (4)
    jc_n = s // P              # key chunks (16)
    nh = GROUPS                # heads on this core (4)
    scale = 1.0 / float(np.sqrt(HD))
    h2 = HD // 2

    nc = bacc.Bacc("TRN2")

    xT_d = nc.dram_tensor("xT", [hid, s], BF, kind="ExternalInput")
    wq = nc.dram_tensor("wq", [hid, nh * HD], BF, kind="ExternalInput")
    wk = nc.dram_tensor("wk", [hid, HD], BF, kind="ExternalInput")
    wv = nc.dram_tensor("wv", [hid, HD], BF, kind="ExternalInput")
    wo = nc.dram_tensor("wo", [nh * HD, hid], BF, kind="ExternalInput")
    cos_d = nc.dram_tensor("cos_t", [P, s], BF, kind="ExternalInput")
    sin_d = nc.dram_tensor("sin_t", [P, s], BF, kind="ExternalInput")
    ident_d = nc.dram_tensor("ident", [P, P], BF, kind="ExternalInput")
    ones_mat_d = nc.dram_tensor("ones_mat", [P, P], BF, kind="ExternalInput")
    out = nc.dram_tensor("out", [s, hid], BF, kind="ExternalOutput")

    xT_r = xT_d.rearrange("(ko ki) s -> ki ko s", ki=P)
    wq_r = wq.rearrange("(ko ki) m -> ki ko m", ki=P)
    wk_r = wk.rearrange("(ko ki) m -> ki ko m", ki=P)
    wv_r = wv.rearrange("(ko ki) m -> ki ko m", ki=P)
    wo_r = wo.rearrange("(h ki) o -> ki h o", ki=P)

    with TileContext(nc) as tc:
        with (
            tc.tile_pool(name="const", bufs=1) as cpool,
            tc.tile_pool(name="kvn", bufs=1) as kvp,
            tc.tile_pool(name="qtp", bufs=2) as qtp,
            tc.tile_pool(name="xt", bufs=2) as xtp,
            tc.tile_pool(name="ropetmp", bufs=2) as rtmp,
            tc.tile_pool(name="vtmp", bufs=2) as vtmpp,
            tc.tile_pool(name="epool", bufs=3) as epool,
            tc.tile_pool(name="tree", bufs=2) as treep,
            tc.tile_pool(name="osb", bufs=2) as osbp,
            tc.tile_pool(name="recip", bufs=2) as rpool,
            tc.tile_pool(name="outsb", bufs=4) as outp,
            tc.tile_pool(name="psQ", bufs=2, space="PSUM") as psQ,
            tc.tile_pool(name="psS", bufs=2, space="PSUM") as psS,
            tc.tile_pool(name="psO", bufs=1, space="PSUM") as psO,
            tc.tile_pool(name="psDen", bufs=1, space="PSUM") as psDen,
        ):
            # ---- constants + weights (issue DMAs early, small first) -------
            ident_bf = cpool.tile([P, P], BF, tag="ident")
            nc.scalar.dma_start(ident_bf[:], ident_d[:, :])
            ones_bf = cpool.tile([P, P], BF, tag="onesm")
            nc.scalar.dma_start(ones_bf[:], ones_mat_d[:, :])
            # wk/wv before the (larger) cos/sin tables: the first K matmuls
            # need wk, while rope only needs cos/sin ~20us later.
            wk_sb = cpool.tile([P, kc_n, HD], BF, tag="wk")
            nc.scalar.dma_start(wk_sb[:, 0:4, :], wk_r[:, 0:4, :])
            nc.scalar.dma_start(wk_sb[:, 4:kc_n, :], wk_r[:, 4:kc_n, :])
            wv_sb = cpool.tile([P, kc_n, HD], BF, tag="wv")
            nc.scalar.dma_start(wv_sb[:], wv_r[:, :, :])
            cos_t = cpool.tile([P, s], BF, tag="cos")
            nc.scalar.dma_start(cos_t[:], cos_d[:, :])
            sin_t = cpool.tile([P, s], BF, tag="sin")
            nc.scalar.dma_start(sin_t[:], sin_d[:, :])
            # wq/wo tiles are declared here but their DMAs are issued later
            # (mid-KV-phase / at main-loop entry) so the x stream wins the
            # HBM bandwidth race during the KV pass.
            wq_sb = cpool.tile([P, kc_n, nh * HD], BF, tag="wq")
            wo_sb = cpool.tile([P, nh, hid], BF, tag="wo")

            kT = kvp.tile([P, s], BF, tag="kT")             # [d, key]
            vnat = kvp.tile([P, jc_n, HD], BF, tag="vn")    # [key, jc, d]

            def load_xt(gs):
                xt = xtp.tile([P, kc_n, SB], BF, tag="xt")
                for c0 in range(0, kc_n, 2):
                    nc.sync.dma_start(
                        xt[:, c0 : c0 + 2, :], xT_r[:, c0 : c0 + 2, gs : gs + SB]
                    )
                return xt

            def rope(dst, src_ps, gs):
                # dst = src * cos + shifted_halves(src) * sin_signed
                t1 = rtmp.tile([P, SB], BF, tag="ropet1")
                t2 = rtmp.tile([P, SB], BF, tag="ropet2")
                nc.vector.tensor_tensor(
                    t1[0:h2, :], src_ps[h2:P, :], sin_t[0:h2, gs : gs + SB], MUL
                )
                nc.vector.tensor_tensor(
                    t1[h2:P, :], src_ps[0:h2, :], sin_t[h2:P, gs : gs + SB], MUL
                )
                nc.vector.tensor_tensor(t2[:], src_ps[:], cos_t[:, gs : gs + SB], MUL)
                nc.vector.tensor_add(dst, t1[:], t2[:])

            # ---- KV pass: K^T + RoPE, V (key-major) for all s-blocks -------
            xt_tiles = {}
            for sb in range(sb_n):
                gs = sb * SB
                xt = load_xt(gs)
                xt_tiles[sb] = xt
                if sb == 2:
                    for c0 in range(0, kc_n, 4):
                        nc.scalar.dma_start(
                            wq_sb[:, c0 : c0 + 4, :], wq_r[:, c0 : c0 + 4, :]
                        )
                k_ps = psQ.tile([P, SB], F32, tag="proj")
                for kc in range(kc_n):
                    nc.tensor.matmul(
                        k_ps[:],
                        wk_sb[:, kc, :],
                        xt[:, kc, :],
                        start=(kc == 0),
                        stop=(kc == kc_n - 1),
                    )
                rope(kT[:, gs : gs + SB], k_ps, gs)

                v_ps = psQ.tile([P, SB], F32, tag="proj")
                for kc in range(kc_n):
                    nc.tensor.matmul(
                        v_ps[:],
                        wv_sb[:, kc, :],
                        xt[:, kc, :],
                        start=(kc == 0),
                        stop=(kc == kc_n - 1),
                    )
                vtmp = vtmpp.tile([P, SB], BF, tag="vtmp")
                nc.scalar.copy(vtmp[:], v_ps[:])
                tps = psS.tile([P, 2, SB], BF, tag="spsum")
                for t in range(SB // P):
                    nc.tensor.transpose(
                        tps[:, 0, t * P : (t + 1) * P],
                        vtmp[:, t * P : (t + 1) * P],
                        ident_bf[:],
                    )
                jc0 = gs // P
                nc.vector.tensor_copy(
                    vnat[:, jc0 : jc0 + SB // P, :],
                    tps[:, 0, :].rearrange("p (a b) -> p a b", a=SB // P),
                )

            # ---- fused main loop ------------------------------------------
            # Per i-block: Qproj+RoPE, then a fine-grained static interleave
            # of score pairs S(h,jp), attn@V pairs A(h-1,jp), and the
            # PREVIOUS i-block's output-projection units, so independent PE
            # work fills every exp-gated stall (the in-order PE queue would
            # otherwise idle ~1us per scores-slot reuse).
            for wh in range(nh):
                nc.scalar.dma_start(wo_sb[:, wh, :], wo_r[:, wh, :])

            def s_pair(h, jp, e_t, qT_blk):
                jc0 = 2 * jp
                s_ps = psS.tile([P, 2, SB], F32, tag="spsum")
                for half in range(2):
                    nc.tensor.matmul(
                        s_ps[:, half, :],
                        kT[:, (jc0 + half) * P : (jc0 + half + 1) * P],
                        qT_blk[:, h, :],
                        start=True,
                        stop=True,
                    )
                nc.scalar.activation(
                    e_t[:, jc0 : jc0 + 2, :], s_ps[:], EXP, scale=scale
                )

            def a_pair(h, jp, e_t, o_ps):
                for jc in (2 * jp, 2 * jp + 1):
                    nc.tensor.matmul(
                        o_ps[:],
                        vnat[:, jc, :],
                        e_t[:, jc, :],
                        start=(jc == 0),
                        stop=(jc == jc_n - 1),
                    )

            def av_finish(h, e_t, o_ps, osb_raw):
                # raw PSUM->SBUF copy then the in-register esum tree on DVE
                nc.vector.tensor_copy(osb_raw[:, h, :], o_ps[:])
                t1 = treep.tile([P, 4, SB], BF, tag="tree", name="t1")
                nc.vector.tensor_add(t1[:], e_t[:, 0:4, :], e_t[:, 4:8, :])
                nc.vector.tensor_add(t1[:], t1[:], e_t[:, 8:12, :])
                nc.vector.tensor_add(t1[:], t1[:], e_t[:, 12:16, :])
                nc.vector.tensor_add(t1[:, 0:2, :], t1[:, 0:2, :], t1[:, 2:4, :])
                nc.vector.tensor_add(t1[:, 0, :], t1[:, 0, :], t1[:, 1, :])
                return t1

            def emit_den(h, t1, osb_raw, osb_t):
                den_ps = psDen.tile([P, SB], F32, tag="den")
                nc.tensor.matmul(
                    den_ps[:], ones_bf[:], t1[:, 0, :], start=True, stop=True
                )
                recip_sb = rpool.tile([P, SB], F32, tag="recipsb")
                nc.vector.reciprocal_approx_fast(recip_sb[:], den_ps[:])
                nc.vector.tensor_tensor(
                    osb_t[:, h, :], osb_raw[:, h, :], recip_sb[:], MUL
                )

            def po_unit(po_gi, po_osb, k, alt_copy=False):
                # one [P,SB] output tile: 4 head-accumulated matmuls on the
                # psQ ring (NOT psS, which is exp-gated), copy, DMA.
                po_ic, po_oc = divmod(k, hid // SB)
                po_ps = psQ.tile([P, SB], F32, tag="proj", name="po_ps")
                for po_h in range(nh):
                    nc.tensor.matmul(
                        po_ps[:],
                        po_osb[:, po_h, po_ic * P : (po_ic + 1) * P],
                        wo_sb[:, po_h, po_oc * SB : (po_oc + 1) * SB],
                        start=(po_h == 0),
                        stop=(po_h == nh - 1),
                    )
                po_out = outp.tile([P, SB], BF, tag="outsb", name="po_out")
                if alt_copy and k % 2 == 0:
                    nc.scalar.copy(po_out[:], po_ps[:])
                else:
                    nc.vector.tensor_copy(po_out[:], po_ps[:])
                nc.sync.dma_start(
                    out[
                        po_gi + po_ic * P : po_gi + (po_ic + 1) * P,
                        po_oc * SB : (po_oc + 1) * SB,
                    ],
                    po_out[:],
                )

            pending = None
            for ib in [3, 2, 1, 0]:
                gi = ib * SB
                # ib3/ib2 reuse the x tiles still resident from the KV pass
                xt = xt_tiles[ib] if ib >= sb_n - 2 else load_xt(gi)
                qT_blk = qtp.tile([P, nh, SB], BF, tag="qT")
                for h in range(nh):
                    q_ps = psQ.tile([P, SB], F32, tag="proj")
                    for kc in range(kc_n):
                        nc.tensor.matmul(
                            q_ps[:],
                            wq_sb[:, kc, h * HD : (h + 1) * HD],
                            xt[:, kc, :],
                            start=(kc == 0),
                            stop=(kc == kc_n - 1),
                        )
                    rope(qT_blk[:, h, :], q_ps, gi)

                osb_t = osbp.tile([P, nh, SB], BF, tag="osb")
                osb_raw = osbp.tile([P, nh, SB], BF, tag="osbraw")
                e_tiles = [None] * nh
                trees = [None] * nh
                o_tiles = [None] * nh
                po_k = [0]

                def po_fill(po2=po_k):
                    if pending is not None and po2[0] < 16:
                        po_unit(pending[0], pending[1], po2[0])
                        po2[0] += 1

                for h in range(nh):
                    e_tiles[h] = epool.tile([P, jc_n, SB], BF, tag="E", name="e_t")
                    if h >= 1:
                        o_tiles[h - 1] = psO.tile(
                            [P, SB], F32, tag="opsum", name="o_ps"
                        )
                    for jp in range(jc_n // 2):
                        s_pair(h, jp, e_tiles[h], qT_blk)
                        if h >= 1:
                            a_pair(h - 1, jp, e_tiles[h - 1], o_tiles[h - 1])
                        if jp % 2 == 1:
                            po_fill()
                        if h >= 2 and jp == 3:
                            emit_den(h - 2, trees[h - 2], osb_raw, osb_t)
                    if h >= 1:
                        trees[h - 1] = av_finish(
                            h - 1, e_tiles[h - 1], o_tiles[h - 1], osb_raw
                        )
                # tail: attn@V for the last head + remaining den work
                o_tiles[nh - 1] = psO.tile([P, SB], F32, tag="opsum", name="o_ps")
                for jp in range(jc_n // 2):
                    a_pair(nh - 1, jp, e_tiles[nh - 1], o_tiles[nh - 1])
                    if jp % 2 == 1:
                        po_fill()
                    if jp == 3:
                        emit_den(nh - 2, trees[nh - 2], osb_raw, osb_t)
                trees[nh - 1] = av_finish(
                    nh - 1, e_tiles[nh - 1], o_tiles[nh - 1], osb_raw
                )
                emit_den(nh - 1, trees[nh - 1], osb_raw, osb_t)

                pending = (gi, osb_t)
            # final output block: exp backlog is drained, so split copies
            # between the Scalar and Vector engines
            for k in range(16):
                po_unit(pending[0], pending[1], k, alt_copy=True)

    nc.finalize()
    return nc


def _get_built():
    global _built
    if _built is None:
        _built = build_bass()
    return _built


def make_in_maps(hidden_states, Wq, Wk, Wv, Wo):
    cos_t, sin_t = rope_tables()
    ident = np.eye(P, dtype=BF16)
    ones_mat = np.ones((P, P), BF16)
    wq_b = Wq.astype(BF16)
    wk_b = Wk.astype(BF16)
    wv_b = Wv.astype(BF16)
    wo_b = Wo.astype(BF16)
    in_maps = []
    xT_b = [np.ascontiguousarray(hidden_states[b].T.astype(BF16)) for b in range(B)]
    for core in range(N_CORES):
        b, g = divmod(core, NKV)
        in_maps.append(
            {
                "xT": xT_b[b],
                "wq": np.ascontiguousarray(wq_b[:, g * GROUPS * HD : (g + 1) * GROUPS * HD]),
                "wk": np.ascontiguousarray(wk_b[:, g * HD : (g + 1) * HD]),
                "wv": np.ascontiguousarray(wv_b[:, g * HD : (g + 1) * HD]),
                "wo": np.ascontiguousarray(wo_b[g * GROUPS * HD : (g + 1) * GROUPS * HD, :]),
                "cos_t": cos_t,
                "sin_t": sin_t,
                "ident": ident,
                "ones_mat": ones_mat,
            }
        )
    return in_maps


def kernel(hidden_states, Wq, Wk, Wv, Wo, trace=False):
    from concourse.bass_utils import run_bass_kernel_spmd

    hidden_states = np.asarray(hidden_states, dtype=np.float32)
    Wq = np.asarray(Wq, dtype=np.float32)
    Wk = np.asarray(Wk, dtype=np.float32)
    Wv = np.asarray(Wv, dtype=np.float32)
    Wo = np.asarray(Wo, dtype=np.float32)

    nc = _get_built()
    in_maps = make_in_maps(hidden_states, Wq, Wk, Wv, Wo)
    res = run_bass_kernel_spmd(nc, in_maps, core_ids=list(range(N_CORES)), trace=trace)

    out = np.zeros((B, S, HID), dtype=np.float32)
    for core in range(N_CORES):
        b = core // NKV
        out[b] += res.results[core]["out"].astype(np.float32)
    if trace:
        kernel.last_result = res
    return out
